# revision 4
# baseline (speedup 1.0000x reference)
"""Trainium2 Bass kernel for nn_ContextModel_85993835200994 — fp8 DoubleRow.

PixelCNN-style context model (see reference):
  out = round(x); masked 5x5 conv (12 taps) 192->384; h=concat(conv,phi) 768
  h1 = leaky(h@w1+b1) 640; h2 = leaky(h1@w2+b2) 640
  cond = h2@w3+b3 = [mean|scale]; lik = Phi((v+.5)/s)-Phi((v-.5)/s)

All matmuls run as fp8e4 DoubleRow (K=256 per matmul, 0.5 cyc/row) with
error compensation: weights are pre-scaled by a per-tensor 2^k (avoids the
e4m3 subnormal floor) and split hi+lo; activations are evacuated to fp16
then split hi+lo on-device. Each layer computes Wh@(Hh+Hl) + Wl@Hh
(~8 effective mantissa bits). x=round(x) is exact in fp8, so the conv
needs only the weight split, done as one broadcast-pair DoubleRow per tap.
The 5-k-tile layers pair the odd k-tile's hi/lo terms in one broadcast DR.
The likelihood runs in "scaled units" (PSUM carries 2^k3 * cond; the 2^k3
cancels between the mean and scale halves), fp16 elementwise, and the
final 0.5x is folded into the host-side gather.

Emission is software-pipelined: conv(c+1) is issued to the PE stream
before mlp1..3(c), so evac+split latency of each stage hides behind
independent matmul work. Elementwise is balanced across ACT/DVE/Pool.

Distribution: data-parallel over batch x image-half -> 8 cores, each
computing a [192, 64, 128] output slice (mode-A conv needs 2 halo rows
above only).
"""

import numpy as np
import ml_dtypes

import concourse.bass as bass
import concourse.mybir as mybir
import concourse.tile as tile
from concourse import bacc
from concourse.bass_utils import run_bass_kernel_spmd

F32 = mybir.dt.float32
F16 = mybir.dt.float16
F8 = mybir.dt.float8e4
AF = mybir.ActivationFunctionType
ALU = mybir.AluOpType
DR = mybir.MatmulPerfMode.DoubleRow
F8NP = ml_dtypes.float8_e4m3
E4MAX = 224.0

C_LAT = 192
C_PHI = 384
HID = 640
B, H, W = 4, 128, 128
N_CORES = 8
ROWS = 64
CHUNKS = [(i * 4, 4) for i in range(15)] + [(60, 2), (62, 2)]
NCH = len(CHUNKS)
XR_H = ROWS + 3
XR_W = W + 6
SQRT2 = 1.4142135623730951

TAPS = [(dy, dx) for dy in (-2, -1) for dx in (-2, -1, 0, 1, 2)] + \
       [(0, -2), (0, -1)]
NT = len(TAPS)
NTK = 18          # conv k-tiles: 12 ch-lo taps + 6 dual-tap ch-hi

TRACE = False
LAST_RESULT = None
_CACHE = {}


def _build(kc, k1, k2, k3):
    nc = bacc.Bacc("TRN2", target_bir_lowering=False, debug=False)

    xr_d = nc.dram_tensor("xr", [C_LAT, XR_H, XR_W], F8, kind="ExternalInput").ap()
    phih_d = nc.dram_tensor("phih", [128, 3, ROWS, W], F8, kind="ExternalInput").ap()
    phil_d = nc.dram_tensor("phil", [128, 3, ROWS, W], F8, kind="ExternalInput").ap()
    wc_d = nc.dram_tensor("wc", [128, NTK, 2, C_PHI], F8, kind="ExternalInput").ap()
    w1h_d = nc.dram_tensor("w1h", [128, 6, HID], F8, kind="ExternalInput").ap()
    w1l_d = nc.dram_tensor("w1l", [128, 6, HID], F8, kind="ExternalInput").ap()
    w2h_d = nc.dram_tensor("w2h", [128, 4, HID], F8, kind="ExternalInput").ap()
    w2l_d = nc.dram_tensor("w2l", [128, 4, HID], F8, kind="ExternalInput").ap()
    w2x_d = nc.dram_tensor("w2x", [128, 2, HID], F8, kind="ExternalInput").ap()
    w3h_d = nc.dram_tensor("w3h", [128, 4, 2 * C_LAT], F8, kind="ExternalInput").ap()
    w3l_d = nc.dram_tensor("w3l", [128, 4, 2 * C_LAT], F8, kind="ExternalInput").ap()
    w3x_d = nc.dram_tensor("w3x", [128, 2, 2 * C_LAT], F8, kind="ExternalInput").ap()
    bc_d = nc.dram_tensor("bc", [128, 3], F32, kind="ExternalInput").ap()
    b1_d = nc.dram_tensor("b1", [128, 5], F32, kind="ExternalInput").ap()
    b2_d = nc.dram_tensor("b2", [128, 5], F32, kind="ExternalInput").ap()
    b3_d = nc.dram_tensor("b3", [128, 8], F32, kind="ExternalInput").ap()
    lik_d = nc.dram_tensor("lik", [C_LAT, ROWS, W], F16, kind="ExternalOutput").ap()

    S = float(2.0 ** k3)
    CLAMP = float(0.11 * SQRT2 * S)

    with tile.TileContext(nc) as tc:
        with tc.tile_pool(name="const", bufs=1) as cpool, \
             tc.tile_pool(name="rp", bufs=4) as rpool, \
             tc.tile_pool(name="hp", bufs=2) as hpool, \
             tc.tile_pool(name="tp", bufs=8) as tpool, \
             tc.tile_pool(name="ps", bufs=8, space="PSUM") as pspool:

            wc_s = cpool.tile([128, NTK, 2, C_PHI], F8, tag="wc")
            w1h_s = cpool.tile([128, 6, HID], F8, tag="w1h")
            w1l_s = cpool.tile([128, 6, HID], F8, tag="w1l")
            w2h_s = cpool.tile([128, 4, HID], F8, tag="w2h")
            w2l_s = cpool.tile([128, 4, HID], F8, tag="w2l")
            w2x_s = cpool.tile([128, 2, HID], F8, tag="w2x")
            w3h_s = cpool.tile([128, 4, 2 * C_LAT], F8, tag="w3h")
            w3l_s = cpool.tile([128, 4, 2 * C_LAT], F8, tag="w3l")
            w3x_s = cpool.tile([128, 2, 2 * C_LAT], F8, tag="w3x")
            bc_s = cpool.tile([128, 3], F32, tag="bc")
            b1_s = cpool.tile([128, 5], F32, tag="b1")
            b2_s = cpool.tile([128, 5], F32, tag="b2")
            b3_s = cpool.tile([128, 8], F32, tag="b3")

            st = {}  # per-chunk tile state

            # split-op engine rotation (13 splits/chunk): slots 0-1 of each
            # mlp stage stay on DVE (they gate the next stage's first DRs);
            # Pool takes late slots only
            v, g = nc.vector, nc.gpsimd
            hi_cycle = [v, g, v,  v, v, v, g, v,  v, v, v, g, v]
            lo_cycle = [v, g, v,  v, v, g, v, g,  v, v, g, v, g]

            def split(h16v, hhv, hlv, idx):
                eng = hi_cycle[idx]
                if eng is nc.scalar:
                    nc.scalar.activation(hhv, h16v, AF.Copy)
                else:
                    eng.tensor_copy(hhv, h16v)
                lo_cycle[idx].tensor_tensor(hlv, h16v, hhv, ALU.subtract)

            def conv_emit(ci):
                y0, rows = CHUNKS[ci]
                N = rows * 128
                nr = rows + 2
                R0 = rpool.tile([128, nr, W + 4], F8, tag="R0")
                if ci == 0:
                    # critical path of the very first matmul: tap-0 weights + R0
                    nc.sync.dma_start(wc_s[:, 0:1], wc_d[:, 0:1])
                nc.sync.dma_start(R0[:, 0:3], xr_d[0:128, y0:y0 + 3, 0:W + 4])
                nc.sync.dma_start(R0[:, 3:nr], xr_d[0:128, y0 + 3:y0 + nr, 0:W + 4])
                if ci == 0:
                    nc.sync.dma_start(wc_s[:, 1:3], wc_d[:, 1:3])
                PT1 = rpool.tile([128, nr, W + 4], F8, tag="PT1")
                nc.sync.dma_start(PT1[0:64], xr_d[128:192, y0:y0 + nr, 0:W + 4])
                nc.sync.dma_start(PT1[64:128], xr_d[128:192, y0 + 1:y0 + nr + 1, 0:W + 4])
                if ci == 0:
                    nc.sync.dma_start(wc_s[:, 3:9], wc_d[:, 3:9])
                PT2 = rpool.tile([128, nr, W + 4], F8, tag="PT2")
                nc.sync.dma_start(PT2[0:64], xr_d[128:192, y0:y0 + nr, 0:W + 4])
                nc.sync.dma_start(PT2[64:128], xr_d[128:192, y0:y0 + nr, 1:W + 5])
                if ci == 0:
                    nc.sync.dma_start(wc_s[:, 9:NTK], wc_d[:, 9:NTK])
                    nc.sync.dma_start(bc_s[:], bc_d)
                Hh = hpool.tile([128, 6, rows, W], F8, tag="Hh")
                Hl = hpool.tile([128, 6, rows, W], F8, tag="Hl")
                nc.sync.dma_start(Hh[:, 0:3], phih_d[:, :, y0:y0 + rows, :])
                nc.sync.dma_start(Hl[:, 0:3], phil_d[:, :, y0:y0 + rows, :])

                pc = [pspool.tile([128, 512], F32, tag="ps", name=f"pc{m}_{ci}")
                      for m in range(3)]
                for t in range(NTK):
                    if t < 12:
                        dy, dx = TAPS[t]
                        src = R0
                    elif t < 17:
                        dy, dx = TAPS[t - 12]
                        src = PT1
                    else:
                        dy, dx = TAPS[10]
                        src = PT2
                    win = src[:, 2 + dy:2 + rows + dy, 2 + dx:2 + dx + W]
                    winb = win.unsqueeze(1).broadcast_to([128, 2, rows, W])
                    for m in range(3):
                        ms = slice(m * 128, (m + 1) * 128)
                        nc.tensor.matmul(pc[m][:, :N], wc_s[:, t, :, ms], winb,
                                         start=(t == 0), stop=(t == NTK - 1),
                                         perf_mode=DR)
                if ci == 0:
                    nc.sync.dma_start(w1h_s[:], w1h_d)
                    nc.sync.dma_start(w1l_s[:], w1l_d)
                    nc.sync.dma_start(b1_s[:], b1_d)

                hc16 = hpool.tile([128, 3, 512], F16, tag="hc16")
                for m in range(3):
                    nc.scalar.activation(hc16[:, m, :N], pc[m][:, :N], AF.Identity,
                                         bias=bc_s[:, m:m + 1],
                                         scale=float(2.0 ** -kc))
                    split(hc16[:, m, :N],
                          Hh[:, 3 + m].rearrange("p r w -> p (r w)"),
                          Hl[:, 3 + m].rearrange("p r w -> p (r w)"), m)
                st[ci] = dict(R0=R0, PT1=PT1, Hh=Hh, Hl=Hl, rows=rows, y0=y0, N=N)

            def mlp_5k(psum_t, whs, wls, wxs, Ah, Al, ms, n):
                """Wh@(Ah+Al) + Wl@Ah over 5 k-tiles; tile 4 via bcast pair;
                tile-4-dependent DRs last (its split lands latest)."""
                a4h = Ah[:, 4, :n].unsqueeze(1).broadcast_to([128, 2, n])
                a4l = Al[:, 4, :n].unsqueeze(1).broadcast_to([128, 2, n])
                seq = [
                    (whs[:, 0:2, ms], Ah[:, 0:2, :n]),
                    (wls[:, 0:2, ms], Ah[:, 0:2, :n]),
                    (whs[:, 0:2, ms], Al[:, 0:2, :n]),
                    (whs[:, 2:4, ms], Ah[:, 2:4, :n]),
                    (wls[:, 2:4, ms], Ah[:, 2:4, :n]),
                    (whs[:, 2:4, ms], Al[:, 2:4, :n]),
                    (wxs[:, :, ms], a4h),
                    (wxs[:, :, ms], a4l),
                ]
                for i, (wv, av) in enumerate(seq):
                    nc.tensor.matmul(psum_t, wv, av, start=(i == 0),
                                     stop=(i == len(seq) - 1), perf_mode=DR)

            def mlp1_emit(ci):
                s = st[ci]
                N = s["N"]
                Hh, Hl = s["Hh"], s["Hl"]
                p1 = [pspool.tile([128, 512], F32, tag="ps", name=f"p1_{m}_{ci}")
                      for m in range(5)]
                for m in range(5):
                    ms = slice(m * 128, (m + 1) * 128)
                    seq = []
                    for j in range(3):
                        seq += [(w1h_s, Hh, j), (w1l_s, Hh, j), (w1h_s, Hl, j)]
                    for i, (ws, hs, j) in enumerate(seq):
                        nc.tensor.matmul(p1[m][:, :N],
                                         ws[:, 2 * j:2 * j + 2, ms],
                                         hs[:, 2 * j:2 * j + 2],
                                         start=(i == 0), stop=(i == len(seq) - 1),
                                         perf_mode=DR)
                h1_16 = hpool.tile([128, 5, 512], F16, tag="h1_16")
                H1h = hpool.tile([128, 5, 512], F8, tag="H1h")
                H1l = hpool.tile([128, 5, 512], F8, tag="H1l")
                for m in range(5):
                    nc.scalar.activation(h1_16[:, m, :N], p1[m][:, :N], AF.Prelu,
                                         bias=b1_s[:, m:m + 1],
                                         scale=float(2.0 ** -k1), alpha=0.01)
                    split(h1_16[:, m, :N], H1h[:, m, :N], H1l[:, m, :N], 3 + m)
                if ci == 0:
                    nc.sync.dma_start(w2h_s[:], w2h_d)
                    nc.sync.dma_start(w2l_s[:], w2l_d)
                    nc.sync.dma_start(w2x_s[:], w2x_d)
                    nc.sync.dma_start(b2_s[:], b2_d)
                s["H1h"], s["H1l"] = H1h, H1l

            def mlp2_emit(ci):
                s = st[ci]
                N = s["N"]
                H1h, H1l = s["H1h"], s["H1l"]
                p2 = [pspool.tile([128, 512], F32, tag="ps", name=f"p2_{m}_{ci}")
                      for m in range(5)]
                for m in range(5):
                    ms = slice(m * 128, (m + 1) * 128)
                    mlp_5k(p2[m][:, :N], w2h_s, w2l_s, w2x_s, H1h, H1l, ms, N)
                h2_16 = hpool.tile([128, 5, 512], F16, tag="h2_16")
                H2h = hpool.tile([128, 5, 512], F8, tag="H2h")
                H2l = hpool.tile([128, 5, 512], F8, tag="H2l")
                for m in range(5):
                    nc.scalar.activation(h2_16[:, m, :N], p2[m][:, :N], AF.Prelu,
                                         bias=b2_s[:, m:m + 1],
                                         scale=float(2.0 ** -k2), alpha=0.01)
                    split(h2_16[:, m, :N], H2h[:, m, :N], H2l[:, m, :N], 8 + m)
                if ci == 0:
                    nc.sync.dma_start(w3h_s[:], w3h_d)
                    nc.sync.dma_start(w3l_s[:], w3l_d)
                    nc.sync.dma_start(w3x_s[:], w3x_d)
                    nc.sync.dma_start(b3_s[:], b3_d)
                s["H2h"], s["H2l"] = H2h, H2l

            def lik_emit(ci, s, g, P, pm, psc, Rg, cpc, cmc, bsc, bsc_base):
                """Likelihood chain for one channel group. `psc` may sit at a
                nonzero PSUM base partition; the ACT Abs evac realigns it to
                base 0 (PSUM in + SB out cross-base is legal)."""
                rows, y0, N = s["rows"], s["y0"], s["N"]
                tg = f"t{g}"
                Rc = Rg[0:P, 2:2 + rows, 2:2 + W]
                # scale chain first: abs -> max -> recip (off mean critical path)
                sabs = tpool.tile([P, 512], F32, tag=tg, name=f"sa{g}_{ci}")
                nc.scalar.activation(sabs[:, :N], psc[:, :N], AF.Abs,
                                     bias=b3_s[bsc_base:bsc_base + P,
                                               bsc:bsc + 1])
                sc = tpool.tile([P, 512], F32, tag=tg, name=f"sc{g}_{ci}")
                sceng = nc.vector if ci >= NCH - 2 else nc.gpsimd
                sceng.tensor_scalar_max(sc[:, :N], sabs[:, :N], CLAMP)
                rq = tpool.tile([P, 512], F32, tag=tg, name=f"rq{g}_{ci}")
                nc.vector.reciprocal_approx_fast(out=rq[:, :N], in_=sc[:, :N])
                tt = tpool.tile([P, 512], F16, tag=tg, name=f"tt{g}_{ci}")
                nc.vector.scalar_tensor_tensor(
                    tt[:, :N], Rc, -S, pm[:, :N], ALU.mult, ALU.add)
                # [em | ep] packed so one Erf covers both halves
                E = tpool.tile([P, 2, 512], F16, tag=tg, name=f"E{g}_{ci}")
                nc.vector.scalar_tensor_tensor(
                    E[:, 0, :N], tt[:, :N], b3_s[0:P, cpc:cpc + 1], rq[:, :N],
                    ALU.add, ALU.mult)
                nc.vector.scalar_tensor_tensor(
                    E[:, 1, :N], tt[:, :N], b3_s[0:P, cmc:cmc + 1], rq[:, :N],
                    ALU.add, ALU.mult)
                E2 = tpool.tile([P, 2, 512], F16, tag=tg, name=f"F{g}_{ci}")
                nc.scalar.activation(E2[:, :, :N], E[:, :, :N], AF.Erf)
                dd = tpool.tile([P, 512], F16, tag=tg, name=f"dd{g}_{ci}")
                ddeng = nc.vector if ci >= NCH - 2 else nc.gpsimd
                ddeng.tensor_tensor(dd[:, :N], E2[:, 0, :N], E2[:, 1, :N],
                                    ALU.subtract)
                ch0 = 0 if g == 0 else 128
                nc.sync.dma_start(lik_d[ch0:ch0 + P, y0:y0 + rows, :],
                                  dd[:, :N])

            def mlp3_emit(ci):
                s = st.pop(ci)
                N = s["N"]
                H2h, H2l, R0, PT1 = s["H2h"], s["H2l"], s["R0"], s["PT1"]
                # 3 M=128 groups, packed [mean128:192|scale128:192],
                # [scale0:128], [mean0:128]; the mixed group goes first so its
                # (small) likelihood chain overlaps the remaining matmuls, and
                # the g0 scale chain overlaps the g0 mean matmuls.
                p3 = []
                for mi in range(3):
                    pt = pspool.tile([128, 512], F32, tag="ps", name=f"p3_{mi}_{ci}")
                    mlp_5k(pt[:, :N], w3h_s, w3l_s, w3x_s, H2h, H2l,
                           slice(mi * 128, (mi + 1) * 128), N)
                    p3.append(pt)
                    if mi == 0:
                        lik_emit(ci, s, 1, 64, pt[0:64], pt[64:128], PT1,
                                 4, 5, 6, 64)
                lik_emit(ci, s, 0, 128, p3[2], p3[1], R0, 0, 1, 2, 0)

            # depth-4 software pipeline: every dependent stage transition has
            # >= a full stage of independent PE work in between. mlp3 is
            # emitted before mlp2 so p3's PSUM banks recycle the conv banks
            # (freed by the first ACT evacs of the iteration).
            for i in range(NCH + 3):
                if i < NCH:
                    conv_emit(i)
                if 1 <= i <= NCH:
                    mlp1_emit(i - 1)
                if i <= NCH:
                    if 3 <= i:
                        mlp3_emit(i - 3)
                    if 2 <= i:
                        mlp2_emit(i - 2)
                else:
                    # drain: keep a stage of distance before each mlp3
                    if i <= NCH + 1:
                        mlp2_emit(i - 2)
                    mlp3_emit(i - 3)

    nc.compile()
    return nc


def _wsplit(wt):
    """per-tensor 2^k scaling + e4m3 hi/lo split. Returns (hi, lo, k)."""
    k = int(np.floor(np.log2(E4MAX / np.abs(wt).max())))
    ws = (wt * (2.0 ** k)).astype(np.float32)
    hi = ws.astype(F8NP)
    lo = (ws - hi.astype(np.float32)).astype(F8NP)
    return hi, lo, k


def _host_pack(mask_w, mask_b, w1, b1, w2, b2, w3, b3):
    wc = np.empty((C_LAT, NT, C_PHI), np.float32)
    for t, (dy, dx) in enumerate(TAPS):
        wc[:, t, :] = mask_w[:, :, dy + 2, dx + 2].T
    wcp = np.empty((128, NTK, C_PHI), np.float32)
    wcp[:, :12] = wc[:128]
    for j in range(6):
        ta, tb = (j, 5 + j) if j < 5 else (10, 11)
        wcp[0:64, 12 + j] = wc[128:, ta]
        wcp[64:128, 12 + j] = wc[128:, tb]
    wch, wcl, kc = _wsplit(wcp)
    wc8 = np.ascontiguousarray(np.stack([wch, wcl], axis=2))

    # w1 rows reordered: slots 0-2 = phi (rows 384:768), 3-5 = conv (0:384)
    w1r = np.concatenate([w1[C_PHI:], w1[:C_PHI]], axis=0)
    w1p = np.ascontiguousarray(w1r.reshape(6, 128, HID).transpose(1, 0, 2))
    w1h, w1l, k1 = _wsplit(w1p)

    w2p = np.ascontiguousarray(w2.reshape(5, 128, HID).transpose(1, 0, 2))
    w2h, w2l, k2 = _wsplit(w2p)
    w2x = np.ascontiguousarray(np.stack([w2h[:, 4], w2l[:, 4]], axis=1))

    w3m = w3.copy()
    w3m[:, C_LAT:] *= SQRT2
    # column order = [mean128:192 | scale128:192] [scale0:128] [mean0:128]
    # so mlp3 runs as 3 full-width M=128 groups (mixed group first)
    perm = np.concatenate([np.arange(128, 192), np.arange(320, 384),
                           np.arange(192, 320), np.arange(0, 128)])
    w3m = np.ascontiguousarray(w3m[:, perm])
    w3p = np.ascontiguousarray(w3m.reshape(5, 128, 2 * C_LAT).transpose(1, 0, 2))
    w3h, w3l, k3 = _wsplit(w3p)
    w3x = np.ascontiguousarray(np.stack([w3h[:, 4], w3l[:, 4]], axis=1))

    S = 2.0 ** k3
    b3pk = np.zeros((128, 8), np.float32)
    b3pk[:, 0] = S * (b3[0:128] + 0.5)
    b3pk[:, 1] = S * (b3[0:128] - 0.5)
    b3pk[:, 2] = S * SQRT2 * b3[192:320]
    b3pk[:64, 4] = S * (b3[128:192] + 0.5)
    b3pk[:64, 5] = S * (b3[128:192] - 0.5)
    # scale bias for ch 128:192 duplicated at both partition halves so the
    # ACT Abs bias is right under either base-alignment convention
    b3pk[:64, 6] = S * SQRT2 * b3[320:384]
    b3pk[64:, 6] = S * SQRT2 * b3[320:384]

    weights = {
        "wc": wc8,
        "w1h": np.ascontiguousarray(w1h), "w1l": np.ascontiguousarray(w1l),
        "w2h": np.ascontiguousarray(w2h[:, :4]),
        "w2l": np.ascontiguousarray(w2l[:, :4]), "w2x": w2x,
        "w3h": np.ascontiguousarray(w3h[:, :4]),
        "w3l": np.ascontiguousarray(w3l[:, :4]), "w3x": w3x,
        "bc": np.ascontiguousarray(mask_b.reshape(3, 128).T),
        "b1": np.ascontiguousarray(b1.reshape(5, 128).T),
        "b2": np.ascontiguousarray(b2.reshape(5, 128).T),
        "b3": b3pk,
    }
    return weights, (kc, k1, k2, k3)


def kernel(x, phi, mask_w, mask_b, w1, b1, w2, b2, w3, b3):
    global LAST_RESULT
    x = np.asarray(x, dtype=np.float32)
    phi = np.asarray(phi, dtype=np.float32)
    weights, ks = _host_pack(
        np.asarray(mask_w, np.float32), np.asarray(mask_b, np.float32),
        np.asarray(w1, np.float32), np.asarray(b1, np.float32),
        np.asarray(w2, np.float32), np.asarray(b2, np.float32),
        np.asarray(w3, np.float32), np.asarray(b3, np.float32))

    R = np.round(x)
    R8 = R.astype(F8NP)
    phih = phi.astype(F8NP)
    phil = (phi - phih.astype(np.float32)).astype(F8NP)

    key = ("nc",) + ks
    if key not in _CACHE:
        _CACHE[key] = _build(*ks)
        _CACHE["nc"] = _CACHE[key]
    nc = _CACHE[key]

    in_maps = []
    for c in range(N_CORES):
        b, r0 = c // 2, (c % 2) * ROWS
        xr_c = np.zeros((C_LAT, XR_H, XR_W), F8NP)
        lo = max(r0 - 2, 0)
        hi = min(r0 + ROWS + 1, H)
        xr_c[:, 2 - (r0 - lo):2 - (r0 - lo) + (hi - lo), 2:2 + W] = R8[b, :, lo:hi, :]
        # phi packed [128, 3, ROWS, W]: partition-major k-tiles
        ph_c = np.ascontiguousarray(
            phih[b, :, r0:r0 + ROWS, :].reshape(3, 128, ROWS, W)
            .transpose(1, 0, 2, 3))
        pl_c = np.ascontiguousarray(
            phil[b, :, r0:r0 + ROWS, :].reshape(3, 128, ROWS, W)
            .transpose(1, 0, 2, 3))
        in_maps.append({"xr": xr_c, "phih": ph_c, "phil": pl_c, **weights})

    res = run_bass_kernel_spmd(nc, in_maps, core_ids=list(range(N_CORES)),
                               trace=TRACE)
    LAST_RESULT = res

    lik = np.empty((B, C_LAT, H, W), np.float32)
    for c in range(N_CORES):
        b, r0 = c // 2, (c % 2) * ROWS
        lik[b, :, r0:r0 + ROWS, :] = \
            np.asarray(res.results[c]["lik"], np.float32) * 0.5
    return R, lik



# revision 5
# speedup vs baseline: 1.3059x; 1.3059x over previous
"""Trainium2 Bass kernel for nn_ContextModel_85993835200994 — fp8 DoubleRow.

PixelCNN-style context model (see reference):
  out = round(x); masked 5x5 conv (12 taps) 192->384; h=concat(conv,phi) 768
  h1 = leaky(h@w1+b1) 640; h2 = leaky(h1@w2+b2) 640
  cond = h2@w3+b3 = [mean|scale]; lik = Phi((v+.5)/s)-Phi((v-.5)/s)

All matmuls run as fp8e4 DoubleRow (K=256 per matmul, 0.5 cyc/row) with
error compensation: weights are pre-scaled by a per-tensor 2^k (avoids the
e4m3 subnormal floor) and split hi+lo; activations are evacuated to fp16
then split hi+lo on-device. Each layer computes Wh@(Hh+Hl) + Wl@Hh
(~8 effective mantissa bits). x=round(x) is exact in fp8, so the conv
needs only the weight split, done as one broadcast-pair DoubleRow per tap.
The 5-k-tile layers pair the odd k-tile's hi/lo terms in one broadcast DR.
The likelihood runs in "scaled units" (PSUM carries 2^k3 * cond; the 2^k3
cancels between the mean and scale halves), fp16 elementwise, and the
final 0.5x is folded into the host-side gather.

Emission is software-pipelined: conv(c+1) is issued to the PE stream
before mlp1..3(c), so evac+split latency of each stage hides behind
independent matmul work. Elementwise is balanced across ACT/DVE/Pool.

Distribution: data-parallel over batch x image-half -> 8 cores, each
computing a [192, 64, 128] output slice (mode-A conv needs 2 halo rows
above only).
"""

import numpy as np
import ml_dtypes

import concourse.bass as bass
import concourse.mybir as mybir
import concourse.tile as tile
from concourse import bacc
from concourse.bass_utils import run_bass_kernel_spmd

F32 = mybir.dt.float32
F16 = mybir.dt.float16
F8 = mybir.dt.float8e4
AF = mybir.ActivationFunctionType
ALU = mybir.AluOpType
DR = mybir.MatmulPerfMode.DoubleRow
F8NP = ml_dtypes.float8_e4m3
E4MAX = 224.0

C_LAT = 192
C_PHI = 384
HID = 640
B, H, W = 4, 128, 128
N_CORES = 8
ROWS = 64
CHUNKS = [(i * 4, 4) for i in range(15)] + [(60, 2), (62, 2)]
NCH = len(CHUNKS)
XR_H = ROWS + 3
XR_W = W + 6
SQRT2 = 1.4142135623730951

TAPS = [(dy, dx) for dy in (-2, -1) for dx in (-2, -1, 0, 1, 2)] + \
       [(0, -2), (0, -1)]
NT = len(TAPS)
NTK = 18          # conv k-tiles: 12 ch-lo taps + 6 dual-tap ch-hi

TRACE = False
LAST_RESULT = None
_CACHE = {}


def _build(kc, k1, k2, k3):
    nc = bacc.Bacc("TRN2", target_bir_lowering=False, debug=False)

    xr_d = nc.dram_tensor("xr", [C_LAT, XR_H, XR_W], F8, kind="ExternalInput").ap()
    phih_d = nc.dram_tensor("phih", [128, 3, ROWS, W], F8, kind="ExternalInput").ap()
    phil_d = nc.dram_tensor("phil", [128, 3, ROWS, W], F8, kind="ExternalInput").ap()
    wc_d = nc.dram_tensor("wc", [128, NTK, 2, C_PHI], F8, kind="ExternalInput").ap()
    w1h_d = nc.dram_tensor("w1h", [128, 6, HID], F8, kind="ExternalInput").ap()
    w1l_d = nc.dram_tensor("w1l", [128, 6, HID], F8, kind="ExternalInput").ap()
    w2h_d = nc.dram_tensor("w2h", [128, 4, HID], F8, kind="ExternalInput").ap()
    w2l_d = nc.dram_tensor("w2l", [128, 4, HID], F8, kind="ExternalInput").ap()
    w2x_d = nc.dram_tensor("w2x", [128, 2, HID], F8, kind="ExternalInput").ap()
    w3h_d = nc.dram_tensor("w3h", [128, 4, 2 * C_LAT], F8, kind="ExternalInput").ap()
    w3l_d = nc.dram_tensor("w3l", [128, 4, 2 * C_LAT], F8, kind="ExternalInput").ap()
    w3x_d = nc.dram_tensor("w3x", [128, 2, 2 * C_LAT], F8, kind="ExternalInput").ap()
    bc_d = nc.dram_tensor("bc", [128, 3], F32, kind="ExternalInput").ap()
    b1_d = nc.dram_tensor("b1", [128, 5], F32, kind="ExternalInput").ap()
    b2_d = nc.dram_tensor("b2", [128, 5], F32, kind="ExternalInput").ap()
    b3_d = nc.dram_tensor("b3", [128, 8], F32, kind="ExternalInput").ap()
    lik_d = nc.dram_tensor("lik", [C_LAT, ROWS, W], F16, kind="ExternalOutput").ap()

    S = float(2.0 ** k3)
    CLAMP = float(0.11 * SQRT2 * S)

    with tile.TileContext(nc) as tc:
        with tc.tile_pool(name="const", bufs=1) as cpool, \
             tc.tile_pool(name="rp", bufs=4) as rpool, \
             tc.tile_pool(name="hp", bufs=2) as hpool, \
             tc.tile_pool(name="tp", bufs=8) as tpool, \
             tc.tile_pool(name="ps", bufs=8, space="PSUM") as pspool:

            wc_s = cpool.tile([128, NTK, 2, C_PHI], F8, tag="wc")
            w1h_s = cpool.tile([128, 6, HID], F8, tag="w1h")
            w1l_s = cpool.tile([128, 6, HID], F8, tag="w1l")
            w2h_s = cpool.tile([128, 4, HID], F8, tag="w2h")
            w2l_s = cpool.tile([128, 4, HID], F8, tag="w2l")
            w2x_s = cpool.tile([128, 2, HID], F8, tag="w2x")
            w3h_s = cpool.tile([128, 4, 2 * C_LAT], F8, tag="w3h")
            w3l_s = cpool.tile([128, 4, 2 * C_LAT], F8, tag="w3l")
            w3x_s = cpool.tile([128, 2, 2 * C_LAT], F8, tag="w3x")
            bc_s = cpool.tile([128, 3], F32, tag="bc")
            b1_s = cpool.tile([128, 5], F32, tag="b1")
            b2_s = cpool.tile([128, 5], F32, tag="b2")
            b3_s = cpool.tile([128, 8], F32, tag="b3")

            st = {}  # per-chunk tile state

            # split-op engine rotation (13 splits/chunk): slots 0-1 of each
            # mlp stage stay on DVE (they gate the next stage's first DRs);
            # Pool takes late slots only
            v, g = nc.vector, nc.gpsimd
            hi_cycle = [v, g, v,  v, v, v, g, v,  v, v, v, g, v]
            lo_cycle = [v, g, v,  v, v, g, v, g,  v, v, g, v, g]

            def split(h16v, hhv, hlv, idx):
                eng = hi_cycle[idx]
                if eng is nc.scalar:
                    nc.scalar.activation(hhv, h16v, AF.Copy)
                else:
                    eng.tensor_copy(hhv, h16v)
                lo_cycle[idx].tensor_tensor(hlv, h16v, hhv, ALU.subtract)

            def conv_emit(ci):
                y0, rows = CHUNKS[ci]
                N = rows * 128
                nr = rows + 2
                R0 = rpool.tile([128, nr, W + 4], F8, tag="R0")
                if ci == 0:
                    # critical path of the very first matmul: tap-0 weights + R0
                    nc.sync.dma_start(wc_s[:, 0:1], wc_d[:, 0:1])
                nc.sync.dma_start(R0[:, 0:3], xr_d[0:128, y0:y0 + 3, 0:W + 4])
                nc.sync.dma_start(R0[:, 3:nr], xr_d[0:128, y0 + 3:y0 + nr, 0:W + 4])
                if ci == 0:
                    nc.sync.dma_start(wc_s[:, 1:3], wc_d[:, 1:3])
                PT1 = rpool.tile([128, nr, W + 4], F8, tag="PT1")
                nc.sync.dma_start(PT1[0:64], xr_d[128:192, y0:y0 + nr, 0:W + 4])
                nc.sync.dma_start(PT1[64:128], xr_d[128:192, y0 + 1:y0 + nr + 1, 0:W + 4])
                if ci == 0:
                    nc.sync.dma_start(wc_s[:, 3:9], wc_d[:, 3:9])
                PT2 = rpool.tile([128, nr, W + 4], F8, tag="PT2")
                nc.sync.dma_start(PT2[0:64], xr_d[128:192, y0:y0 + nr, 0:W + 4])
                nc.sync.dma_start(PT2[64:128], xr_d[128:192, y0:y0 + nr, 1:W + 5])
                if ci == 0:
                    nc.sync.dma_start(wc_s[:, 9:NTK], wc_d[:, 9:NTK])
                    nc.sync.dma_start(bc_s[:], bc_d)
                Hh = hpool.tile([128, 6, rows, W], F8, tag="Hh")
                Hl = hpool.tile([128, 6, rows, W], F8, tag="Hl")
                nc.sync.dma_start(Hh[:, 0:3], phih_d[:, :, y0:y0 + rows, :])
                nc.sync.dma_start(Hl[:, 0:3], phil_d[:, :, y0:y0 + rows, :])

                pc = [pspool.tile([128, 512], F32, tag="ps", name=f"pc{m}_{ci}")
                      for m in range(3)]
                for t in range(NTK):
                    if t < 12:
                        dy, dx = TAPS[t]
                        src = R0
                    elif t < 17:
                        dy, dx = TAPS[t - 12]
                        src = PT1
                    else:
                        dy, dx = TAPS[10]
                        src = PT2
                    win = src[:, 2 + dy:2 + rows + dy, 2 + dx:2 + dx + W]
                    winb = win.unsqueeze(1).broadcast_to([128, 2, rows, W])
                    for m in range(3):
                        ms = slice(m * 128, (m + 1) * 128)
                        nc.tensor.matmul(pc[m][:, :N], wc_s[:, t, :, ms], winb,
                                         start=(t == 0), stop=(t == NTK - 1),
                                         perf_mode=DR)
                if ci == 0:
                    nc.sync.dma_start(w1h_s[:], w1h_d)
                    nc.sync.dma_start(w1l_s[:], w1l_d)
                    nc.sync.dma_start(b1_s[:], b1_d)

                hc16 = hpool.tile([128, 3, 512], F16, tag="hc16")
                for m in range(3):
                    nc.scalar.activation(hc16[:, m, :N], pc[m][:, :N], AF.Identity,
                                         bias=bc_s[:, m:m + 1],
                                         scale=float(2.0 ** -kc))
                    split(hc16[:, m, :N],
                          Hh[:, 3 + m].rearrange("p r w -> p (r w)"),
                          Hl[:, 3 + m].rearrange("p r w -> p (r w)"), m)
                st[ci] = dict(R0=R0, PT1=PT1, Hh=Hh, Hl=Hl, rows=rows, y0=y0, N=N)

            def mlp_5k(psum_t, whs, wls, wxs, Ah, Al, ms, n):
                """Wh@(Ah+Al) + Wl@Ah over 5 k-tiles; tile 4 via bcast pair;
                tile-4-dependent DRs last (its split lands latest)."""
                a4h = Ah[:, 4, :n].unsqueeze(1).broadcast_to([128, 2, n])
                a4l = Al[:, 4, :n].unsqueeze(1).broadcast_to([128, 2, n])
                seq = [
                    (whs[:, 0:2, ms], Ah[:, 0:2, :n]),
                    (wls[:, 0:2, ms], Ah[:, 0:2, :n]),
                    (whs[:, 0:2, ms], Al[:, 0:2, :n]),
                    (whs[:, 2:4, ms], Ah[:, 2:4, :n]),
                    (wls[:, 2:4, ms], Ah[:, 2:4, :n]),
                    (whs[:, 2:4, ms], Al[:, 2:4, :n]),
                    (wxs[:, :, ms], a4h),
                    (wxs[:, :, ms], a4l),
                ]
                for i, (wv, av) in enumerate(seq):
                    nc.tensor.matmul(psum_t, wv, av, start=(i == 0),
                                     stop=(i == len(seq) - 1), perf_mode=DR)

            def mlp1_emit(ci):
                s = st[ci]
                N = s["N"]
                Hh, Hl = s["Hh"], s["Hl"]
                p1 = [pspool.tile([128, 512], F32, tag="ps", name=f"p1_{m}_{ci}")
                      for m in range(5)]
                for m in range(5):
                    ms = slice(m * 128, (m + 1) * 128)
                    seq = []
                    for j in range(3):
                        seq += [(w1h_s, Hh, j), (w1l_s, Hh, j), (w1h_s, Hl, j)]
                    for i, (ws, hs, j) in enumerate(seq):
                        nc.tensor.matmul(p1[m][:, :N],
                                         ws[:, 2 * j:2 * j + 2, ms],
                                         hs[:, 2 * j:2 * j + 2],
                                         start=(i == 0), stop=(i == len(seq) - 1),
                                         perf_mode=DR)
                h1_16 = hpool.tile([128, 5, 512], F16, tag="h1_16")
                H1h = hpool.tile([128, 5, 512], F8, tag="H1h")
                H1l = hpool.tile([128, 5, 512], F8, tag="H1l")
                for m in range(5):
                    nc.scalar.activation(h1_16[:, m, :N], p1[m][:, :N], AF.Prelu,
                                         bias=b1_s[:, m:m + 1],
                                         scale=float(2.0 ** -k1), alpha=0.01)
                    split(h1_16[:, m, :N], H1h[:, m, :N], H1l[:, m, :N], 3 + m)
                if ci == 0:
                    nc.sync.dma_start(w2h_s[:], w2h_d)
                    nc.sync.dma_start(w2l_s[:], w2l_d)
                    nc.sync.dma_start(w2x_s[:], w2x_d)
                    nc.sync.dma_start(b2_s[:], b2_d)
                s["H1h"], s["H1l"] = H1h, H1l

            def mlp2_emit(ci):
                s = st[ci]
                N = s["N"]
                H1h, H1l = s["H1h"], s["H1l"]
                p2 = [pspool.tile([128, 512], F32, tag="ps", name=f"p2_{m}_{ci}")
                      for m in range(5)]
                for m in range(5):
                    ms = slice(m * 128, (m + 1) * 128)
                    mlp_5k(p2[m][:, :N], w2h_s, w2l_s, w2x_s, H1h, H1l, ms, N)
                h2_16 = hpool.tile([128, 5, 512], F16, tag="h2_16")
                H2h = hpool.tile([128, 5, 512], F8, tag="H2h")
                H2l = hpool.tile([128, 5, 512], F8, tag="H2l")
                for m in range(5):
                    nc.scalar.activation(h2_16[:, m, :N], p2[m][:, :N], AF.Prelu,
                                         bias=b2_s[:, m:m + 1],
                                         scale=float(2.0 ** -k2), alpha=0.01)
                    split(h2_16[:, m, :N], H2h[:, m, :N], H2l[:, m, :N], 8 + m)
                if ci == 0:
                    nc.sync.dma_start(w3h_s[:], w3h_d)
                    nc.sync.dma_start(w3l_s[:], w3l_d)
                    nc.sync.dma_start(w3x_s[:], w3x_d)
                    nc.sync.dma_start(b3_s[:], b3_d)
                s["H2h"], s["H2l"] = H2h, H2l

            def lik_emit(ci, s, g, P, pm, psc, Rg, cpc, cmc, bsc, bsc_base):
                """Likelihood chain for one channel group. `psc` may sit at a
                nonzero PSUM base partition; the ACT Abs evac realigns it to
                base 0 (PSUM in + SB out cross-base is legal)."""
                rows, y0, N = s["rows"], s["y0"], s["N"]
                tg = f"t{g}"
                Rc = Rg[0:P, 2:2 + rows, 2:2 + W]
                # scale chain first: abs -> max -> recip (off mean critical path)
                sabs = tpool.tile([P, 512], F32, tag=tg, name=f"sa{g}_{ci}")
                nc.scalar.activation(sabs[:, :N], psc[:, :N], AF.Abs,
                                     bias=b3_s[bsc_base:bsc_base + P,
                                               bsc:bsc + 1])
                sc = tpool.tile([P, 512], F32, tag=tg, name=f"sc{g}_{ci}")
                sceng = nc.vector if ci >= NCH - 2 else nc.gpsimd
                sceng.tensor_scalar_max(sc[:, :N], sabs[:, :N], CLAMP)
                rq = tpool.tile([P, 512], F32, tag=tg, name=f"rq{g}_{ci}")
                nc.vector.reciprocal_approx_fast(out=rq[:, :N], in_=sc[:, :N])
                tt = tpool.tile([P, 512], F16, tag=tg, name=f"tt{g}_{ci}")
                nc.vector.scalar_tensor_tensor(
                    tt[:, :N], Rc, -S, pm[:, :N], ALU.mult, ALU.add)
                # [em | ep] packed so one Erf covers both halves
                E = tpool.tile([P, 2, 512], F16, tag=tg, name=f"E{g}_{ci}")
                nc.vector.scalar_tensor_tensor(
                    E[:, 0, :N], tt[:, :N], b3_s[0:P, cpc:cpc + 1], rq[:, :N],
                    ALU.add, ALU.mult)
                nc.vector.scalar_tensor_tensor(
                    E[:, 1, :N], tt[:, :N], b3_s[0:P, cmc:cmc + 1], rq[:, :N],
                    ALU.add, ALU.mult)
                E2 = tpool.tile([P, 2, 512], F16, tag=tg, name=f"F{g}_{ci}")
                nc.scalar.activation(E2[:, :, :N], E[:, :, :N], AF.Erf)
                dd = tpool.tile([P, 512], F16, tag=tg, name=f"dd{g}_{ci}")
                ddeng = nc.vector if ci >= NCH - 2 else nc.gpsimd
                ddeng.tensor_tensor(dd[:, :N], E2[:, 0, :N], E2[:, 1, :N],
                                    ALU.subtract)
                ch0 = 0 if g == 0 else 128
                nc.sync.dma_start(lik_d[ch0:ch0 + P, y0:y0 + rows, :],
                                  dd[:, :N])

            def mlp3_emit(ci):
                s = st.pop(ci)
                N = s["N"]
                H2h, H2l, R0, PT1 = s["H2h"], s["H2l"], s["R0"], s["PT1"]
                # 3 M=128 groups, packed [mean128:192|scale128:192],
                # [scale0:128], [mean0:128]; the mixed group goes first so its
                # (small) likelihood chain overlaps the remaining matmuls, and
                # the g0 scale chain overlaps the g0 mean matmuls.
                p3 = []
                for mi in range(3):
                    pt = pspool.tile([128, 512], F32, tag="ps", name=f"p3_{mi}_{ci}")
                    mlp_5k(pt[:, :N], w3h_s, w3l_s, w3x_s, H2h, H2l,
                           slice(mi * 128, (mi + 1) * 128), N)
                    p3.append(pt)
                    if mi == 0:
                        lik_emit(ci, s, 1, 64, pt[0:64], pt[64:128], PT1,
                                 4, 5, 6, 64)
                lik_emit(ci, s, 0, 128, p3[2], p3[1], R0, 0, 1, 2, 0)

            # depth-4 software pipeline: every dependent stage transition has
            # >= a full stage of independent PE work in between. mlp3 leads
            # each iteration so its likelihood chain ops sit at the head of
            # the engine queues: p3 PSUM banks free fast, and every 8-back
            # psum-pool pairing lands on an ACT-evac-freed tile.
            for i in range(NCH + 3):
                if 3 <= i:
                    mlp3_emit(i - 3)
                if i < NCH:
                    conv_emit(i)
                if 1 <= i <= NCH:
                    mlp1_emit(i - 1)
                if 2 <= i <= NCH + 1:
                    mlp2_emit(i - 2)

    nc.compile()
    return nc


def _wsplit(wt):
    """per-tensor 2^k scaling + e4m3 hi/lo split. Returns (hi, lo, k)."""
    k = int(np.floor(np.log2(E4MAX / np.abs(wt).max())))
    ws = (wt * (2.0 ** k)).astype(np.float32)
    hi = ws.astype(F8NP)
    lo = (ws - hi.astype(np.float32)).astype(F8NP)
    return hi, lo, k


def _host_pack(mask_w, mask_b, w1, b1, w2, b2, w3, b3):
    wc = np.empty((C_LAT, NT, C_PHI), np.float32)
    for t, (dy, dx) in enumerate(TAPS):
        wc[:, t, :] = mask_w[:, :, dy + 2, dx + 2].T
    wcp = np.empty((128, NTK, C_PHI), np.float32)
    wcp[:, :12] = wc[:128]
    for j in range(6):
        ta, tb = (j, 5 + j) if j < 5 else (10, 11)
        wcp[0:64, 12 + j] = wc[128:, ta]
        wcp[64:128, 12 + j] = wc[128:, tb]
    wch, wcl, kc = _wsplit(wcp)
    wc8 = np.ascontiguousarray(np.stack([wch, wcl], axis=2))

    # w1 rows reordered: slots 0-2 = phi (rows 384:768), 3-5 = conv (0:384)
    w1r = np.concatenate([w1[C_PHI:], w1[:C_PHI]], axis=0)
    w1p = np.ascontiguousarray(w1r.reshape(6, 128, HID).transpose(1, 0, 2))
    w1h, w1l, k1 = _wsplit(w1p)

    w2p = np.ascontiguousarray(w2.reshape(5, 128, HID).transpose(1, 0, 2))
    w2h, w2l, k2 = _wsplit(w2p)
    w2x = np.ascontiguousarray(np.stack([w2h[:, 4], w2l[:, 4]], axis=1))

    w3m = w3.copy()
    w3m[:, C_LAT:] *= SQRT2
    # column order = [mean128:192 | scale128:192] [scale0:128] [mean0:128]
    # so mlp3 runs as 3 full-width M=128 groups (mixed group first)
    perm = np.concatenate([np.arange(128, 192), np.arange(320, 384),
                           np.arange(192, 320), np.arange(0, 128)])
    w3m = np.ascontiguousarray(w3m[:, perm])
    w3p = np.ascontiguousarray(w3m.reshape(5, 128, 2 * C_LAT).transpose(1, 0, 2))
    w3h, w3l, k3 = _wsplit(w3p)
    w3x = np.ascontiguousarray(np.stack([w3h[:, 4], w3l[:, 4]], axis=1))

    S = 2.0 ** k3
    b3pk = np.zeros((128, 8), np.float32)
    b3pk[:, 0] = S * (b3[0:128] + 0.5)
    b3pk[:, 1] = S * (b3[0:128] - 0.5)
    b3pk[:, 2] = S * SQRT2 * b3[192:320]
    b3pk[:64, 4] = S * (b3[128:192] + 0.5)
    b3pk[:64, 5] = S * (b3[128:192] - 0.5)
    # scale bias for ch 128:192 duplicated at both partition halves so the
    # ACT Abs bias is right under either base-alignment convention
    b3pk[:64, 6] = S * SQRT2 * b3[320:384]
    b3pk[64:, 6] = S * SQRT2 * b3[320:384]

    weights = {
        "wc": wc8,
        "w1h": np.ascontiguousarray(w1h), "w1l": np.ascontiguousarray(w1l),
        "w2h": np.ascontiguousarray(w2h[:, :4]),
        "w2l": np.ascontiguousarray(w2l[:, :4]), "w2x": w2x,
        "w3h": np.ascontiguousarray(w3h[:, :4]),
        "w3l": np.ascontiguousarray(w3l[:, :4]), "w3x": w3x,
        "bc": np.ascontiguousarray(mask_b.reshape(3, 128).T),
        "b1": np.ascontiguousarray(b1.reshape(5, 128).T),
        "b2": np.ascontiguousarray(b2.reshape(5, 128).T),
        "b3": b3pk,
    }
    return weights, (kc, k1, k2, k3)


def kernel(x, phi, mask_w, mask_b, w1, b1, w2, b2, w3, b3):
    global LAST_RESULT
    x = np.asarray(x, dtype=np.float32)
    phi = np.asarray(phi, dtype=np.float32)
    weights, ks = _host_pack(
        np.asarray(mask_w, np.float32), np.asarray(mask_b, np.float32),
        np.asarray(w1, np.float32), np.asarray(b1, np.float32),
        np.asarray(w2, np.float32), np.asarray(b2, np.float32),
        np.asarray(w3, np.float32), np.asarray(b3, np.float32))

    R = np.round(x)
    R8 = R.astype(F8NP)
    phih = phi.astype(F8NP)
    phil = (phi - phih.astype(np.float32)).astype(F8NP)

    key = ("nc",) + ks
    if key not in _CACHE:
        _CACHE[key] = _build(*ks)
        _CACHE["nc"] = _CACHE[key]
    nc = _CACHE[key]

    in_maps = []
    for c in range(N_CORES):
        b, r0 = c // 2, (c % 2) * ROWS
        xr_c = np.zeros((C_LAT, XR_H, XR_W), F8NP)
        lo = max(r0 - 2, 0)
        hi = min(r0 + ROWS + 1, H)
        xr_c[:, 2 - (r0 - lo):2 - (r0 - lo) + (hi - lo), 2:2 + W] = R8[b, :, lo:hi, :]
        # phi packed [128, 3, ROWS, W]: partition-major k-tiles
        ph_c = np.ascontiguousarray(
            phih[b, :, r0:r0 + ROWS, :].reshape(3, 128, ROWS, W)
            .transpose(1, 0, 2, 3))
        pl_c = np.ascontiguousarray(
            phil[b, :, r0:r0 + ROWS, :].reshape(3, 128, ROWS, W)
            .transpose(1, 0, 2, 3))
        in_maps.append({"xr": xr_c, "phih": ph_c, "phil": pl_c, **weights})

    res = run_bass_kernel_spmd(nc, in_maps, core_ids=list(range(N_CORES)),
                               trace=TRACE)
    LAST_RESULT = res

    lik = np.empty((B, C_LAT, H, W), np.float32)
    for c in range(N_CORES):
        b, r0 = c // 2, (c % 2) * ROWS
        lik[b, :, r0:r0 + ROWS, :] = \
            np.asarray(res.results[c]["lik"], np.float32) * 0.5
    return R, lik

